# revision 1
# baseline (speedup 1.0000x reference)
"""Trainium2 Bass kernel for nn_MessagePassing (gnn_message_passing).

Self-contained: takes full (unsharded) numpy inputs, shards batch*rounds
across 8 NeuronCores, runs a Bass/Tile kernel per core, gathers the full
output.

Math (per (b,r) group, all biases included):
  q      = Wq @ ques + bq                       [H]
  edges  = W1a @ on + W1b @ adj + b1            [H, N*E]  (on broadcast over E)
  a      = softmax_E(We @ (q*edges) + be)       -> folded:  (We*diag(q)) @ edges
  edges2 = a * edges
  t      = W2a @ adj + W2b @ edges2 + b2
  b      = softmax_E(Wv @ (q*t) + bv)           -> folded:  (Wv*diag(q)) @ t
  out    = sum_E b * (Wadj @ adj + badj)        [H, N]

Layout on device: hidden channels on partitions (4 chunks of 128), tokens
(node*E+e) on the free dim, so softmax over E is a free-dim segment reduce.

Schedule: groups are software-pipelined — the front half (loads, q-fold,
edges, softmax-a chain) of group g is emitted before the back half
(t/softmax-b/output) of group g-1, so the PE always has a ready matmul
stage while the DVE/GPSIMD softmax chain of the newer group runs.
"""

import os
import sys

for _p in ("/opt/trn_rl_repo", "/root/.axon_site/_ro/trn_rl_repo",
           "/root/.axon_site/_ro/pypackages"):
    if _p not in sys.path and os.path.isdir(_p):
        sys.path.append(_p)

import contextlib
import ctypes
import types

import ml_dtypes
import numpy as np

import concourse.bass as bass
import concourse.tile as tile
from concourse import mybir

BF = mybir.dt.bfloat16
F32 = mybir.dt.float32
AX = mybir.AxisListType
ALU = mybir.AluOpType
ACTF = mybir.ActivationFunctionType

B, R, N, E, D, H = 4, 10, 80, 20, 300, 512
BR = B * R              # 40 (b,r) groups
NCORES = 8
G = BR // NCORES        # 5 groups per core
TOK = N * E             # 1600 tokens per group
NT = 4                  # token tiles per group
T = TOK // NT           # 400 tokens per tile
TN = N // NT            # 20 nodes per tile

KD = [(0, 128), (128, 256), (256, 300)]               # D=300 contraction chunks
KH = [(0, 128), (128, 256), (256, 384), (384, 512)]   # H=512 contraction chunks
MS = [(0, 128), (128, 256), (256, 384), (384, 512)]   # output chunks

_MAXW = 1  # this walrus build allows a single semaphore wait per instruction


def _split_multi_waits(nc):
    """Walrus here rejects instructions with >1 sem wait; hoist extra waits
    onto same-engine NoOps inserted just before the instruction."""
    ctr = 0
    for fn in nc.m.functions:
        for bb in fn.blocks:
            new = []
            for inst in bb.instructions:
                si = inst.sync_info
                if si is not None:
                    waits = list(si.on_wait)
                    if len(waits) > _MAXW:
                        for i in range(0, len(waits) - _MAXW, _MAXW):
                            ctr += 1
                            nop = mybir.InstNoOp(name=f"wsplit-{ctr}")
                            nop.engine = inst.engine
                            nop.sync_info = mybir.SyncInfo(
                                on_wait=waits[i : i + _MAXW], on_update=[]
                            )
                            new.append(nop)
                        si.on_wait = waits[len(waits) - _MAXW :]
                new.append(inst)
            bb.instructions = new
    return ctr


def _patch_ldw_dedupe():
    """The bass pipeline splits every matmul into Ldweights + Matmult.
    Consecutive matmuls that share the stationary operand then reload the
    same weights. Drop the redundant Ldweights at the BIR-JSON level
    (walrus's own --enable-ldw-opt rejects explicit Ldweights)."""
    import orjson

    import concourse.bass2jax as b2j
    import concourse.bass_utils as bu

    if getattr(bu, "_ldw_dedupe_patched", False):
        return
    orig = bu.compile_bir_kernel

    def _dedupe(bir_json):
        d = orjson.loads(bir_json)
        removed = 0
        nopctr = 0
        for fn in d.get("functions", []):
            stack = list(fn.get("blocks", []))
            while stack:
                blk = stack.pop()
                stack.extend(blk.get("blocks", []))
                insts = blk.get("instructions", [])
                out = []
                last_key = None
                for i in insts:
                    op = i.get("opcode")
                    if op == "Ldweights":
                        key = orjson.dumps(
                            [
                                i.get("ins"),
                                i.get("perf_mode"),
                                i.get("tile_position"),
                                i.get("tile_size"),
                                i.get("is_transpose"),
                            ]
                        )
                        si = i.get("sync_info") or {}
                        if key == last_key and not si.get("on_update"):
                            w = si.get("on_wait") or []
                            if w:
                                nopctr += 1
                                out.append(
                                    {
                                        "name": f"ldwkeep-{nopctr}",
                                        "opcode": "NoOp",
                                        "engine": i.get("engine", "PE"),
                                        "ins": [],
                                        "outs": [],
                                        "sync_info": {
                                            "on_wait": w,
                                            "on_update": [],
                                        },
                                    }
                                )
                            removed += 1
                            continue
                        last_key = key
                    elif op == "Matmult":
                        if i.get("is_transpose") or i.get("ldweights"):
                            last_key = None
                    out.append(i)
                blk["instructions"] = out
        if os.environ.get("KERNEL_DEBUG"):
            print(f"ldw dedupe: removed {removed}", file=sys.stderr)
        return orjson.dumps(d)

    def compile_bir_kernel(bir_json, tmpdir, neff_name="file.neff"):
        try:
            bir_json = _dedupe(bir_json)
        except Exception as e:  # pragma: no cover - safety net
            print(f"ldw dedupe skipped: {e}", file=sys.stderr)
        return orig(bir_json, tmpdir, neff_name=neff_name)

    bu.compile_bir_kernel = compile_bir_kernel
    b2j.compile_bir_kernel = compile_bir_kernel
    bu._ldw_dedupe_patched = True


def _install_ntff_hook():
    """Provide antenv.axon_hooks (missing in this image) so that
    run_bass_kernel_spmd(trace=True) can profile via libaxon_pjrt."""
    if "antenv.axon_hooks" in sys.modules:
        return

    def _mk(so_path):
        try:
            lib = ctypes.CDLL(so_path)
        except OSError:
            return None
        if not hasattr(lib, "axon_start_nrt_profile"):
            return None
        lib.axon_start_nrt_profile.argtypes = [
            ctypes.POINTER(ctypes.c_int64),
            ctypes.c_size_t,
        ]
        lib.axon_start_nrt_profile.restype = ctypes.c_int64
        lib.axon_stop_nrt_profile.argtypes = [ctypes.c_char_p]
        lib.axon_stop_nrt_profile.restype = ctypes.c_int64

        @contextlib.contextmanager
        def _hook(output_dir, device_ids):
            import jax

            jax.devices()
            if device_ids:
                ids = (ctypes.c_int64 * len(device_ids))(*device_ids)
                rc = lib.axon_start_nrt_profile(ids, len(device_ids))
            else:
                rc = lib.axon_start_nrt_profile(None, 0)
            if rc != 0:
                raise RuntimeError(f"axon_start_nrt_profile rc={rc}")
            try:
                yield
            finally:
                n = lib.axon_stop_nrt_profile(str(output_dir).encode())
                print(f"ntff profile: {n} file(s) -> {output_dir}", file=sys.stderr)

        return _hook

    hook = _mk("/opt/axon/libaxon_pjrt.so")
    mod = types.ModuleType("antenv.axon_hooks")
    mod.get_axon_ntff_profile_hook = lambda: hook
    try:
        import antenv

        antenv.axon_hooks = mod
    except ImportError:
        pass
    sys.modules["antenv.axon_hooks"] = mod

    import concourse.bass_utils as bass_utils

    bass_utils.upload_artifacts = lambda tmpdir: f"local://{tmpdir}"


def _re3(ap):
    """[128, n*E] -> [128, n, E] view."""
    return ap.rearrange("p (n e) -> p n e", e=E)


def build_program():
    nc = bass.Bass()

    adjT = nc.declare_dram_parameter("adjT", [G, D, TOK], BF, isOutput=False)
    onT = nc.declare_dram_parameter("onT", [G, D, N], BF, isOutput=False)
    quesT = nc.declare_dram_parameter("quesT", [G, 128, 4], BF, isOutput=False)
    w1a_d = nc.declare_dram_parameter("w1a", [D, H], BF, isOutput=False)
    w1b_d = nc.declare_dram_parameter("w1b", [D, H], BF, isOutput=False)
    w2a_d = nc.declare_dram_parameter("w2a", [D, H], BF, isOutput=False)
    wadj_d = nc.declare_dram_parameter("wadj", [D, H], BF, isOutput=False)
    wq_d = nc.declare_dram_parameter("wq", [H, H], BF, isOutput=False)
    we_d = nc.declare_dram_parameter("we", [H, H], BF, isOutput=False)
    w2b_d = nc.declare_dram_parameter("w2b", [H, H], BF, isOutput=False)
    wv_d = nc.declare_dram_parameter("wv", [H, H], BF, isOutput=False)
    # biases packed [128, 4] (column j = channels j*128..j*128+127)
    bq_d = nc.declare_dram_parameter("bq", [128, 4], F32, isOutput=False)
    b1row_d = nc.declare_dram_parameter("b1row", [1, H], BF, isOutput=False)
    smat_d = nc.declare_dram_parameter("smat", [N + 1, TOK], BF, isOutput=False)
    be_d = nc.declare_dram_parameter("be", [128, 4], F32, isOutput=False)
    b2_d = nc.declare_dram_parameter("b2", [128, 4], F32, isOutput=False)
    bv_d = nc.declare_dram_parameter("bv", [128, 4], F32, isOutput=False)
    badj_d = nc.declare_dram_parameter("badj", [128, 4], F32, isOutput=False)

    outT = nc.declare_dram_parameter("outT", [G, 4, 128, N], F32, isOutput=True)

    def tsl(t):
        return slice(t * T, (t + 1) * T)

    with tile.TileContext(nc) as tc, contextlib.ExitStack() as ctx:
        wpool = ctx.enter_context(tc.tile_pool(name="weights", bufs=1))
        gpool = ctx.enter_context(tc.tile_pool(name="group", bufs=2))
        gpool3 = ctx.enter_context(tc.tile_pool(name="group3", bufs=3))
        spool = ctx.enter_context(tc.tile_pool(name="small", bufs=2))
        pspool = ctx.enter_context(tc.tile_pool(name="ps", bufs=8, space="PSUM"))

        # PE warmup: keep the HAM clock-gate at 8/8 through the startup
        # DMA wait so the first real matmuls run at 2.4 GHz.
        wu_sb = wpool.tile([128, 512], BF, tag="wu", name="wu")
        nc.vector.memset(wu_sb[:], 0.0)
        wu_ps = pspool.tile([128, T], F32, tag="ps", name="wups")
        for i in range(85):
            nc.tensor.matmul(
                wu_ps[:], wu_sb[:, :128], wu_sb[:, :T], start=True, stop=True
            )

        def load_w_chunks(dram, chunks, name):
            tiles = []
            for ki, (k0, k1) in enumerate(chunks):
                t_ = wpool.tile(
                    [k1 - k0, H], BF, tag=f"{name}{ki}", name=f"{name}{ki}"
                )
                nc.scalar.dma_start(out=t_[:], in_=dram[k0:k1, :])
                tiles.append(t_)
            return tiles

        w1a_sb = load_w_chunks(w1a_d, KD, "w1a")
        w1b_sb = load_w_chunks(w1b_d, KD, "w1b")
        w2a_sb = load_w_chunks(w2a_d, KD, "w2a")
        wadj_sb = load_w_chunks(wadj_d, KD, "wadj")
        wq_sb = load_w_chunks(wq_d, KH, "wq")
        we_sb = load_w_chunks(we_d, KH, "we")
        w2b_sb = load_w_chunks(w2b_d, KH, "w2b")
        wv_sb = load_w_chunks(wv_d, KH, "wv")

        def load_bias(dram, name):
            t_ = wpool.tile([128, 4], F32, tag=name, name=name)
            nc.scalar.dma_start(out=t_[:], in_=dram[:, :])
            return t_

        bq_sb = load_bias(bq_d, "bq")
        be_sb = load_bias(be_d, "be")
        b2_sb = load_bias(b2_d, "b2")
        bv_sb = load_bias(bv_d, "bv")
        badj_sb = load_bias(badj_d, "badj")

        def emit_preamble(g):
            """Loads + q-fold + on-term (+ its E-expansion)."""
            st = {}
            ques_sb = spool.tile([128, 4], BF, tag="ques", name=f"ques_{g}")
            nc.sync.dma_start(out=ques_sb[:], in_=quesT[g, :, :])
            adj_sb = []
            for ki, (k0, k1) in enumerate(KD):
                t_ = gpool3.tile(
                    [k1 - k0, TOK], BF, tag=f"adj{ki}", name=f"adj{ki}_{g}"
                )
                nc.sync.dma_start(out=t_[:], in_=adjT[g, k0:k1, :])
                adj_sb.append(t_)
            on_sb = []
            for ki, (k0, k1) in enumerate(KD):
                t_ = spool.tile(
                    [k1 - k0, N], BF, tag=f"on{ki}", name=f"on{ki}_{g}"
                )
                nc.sync.dma_start(out=t_[:], in_=onT[g, k0:k1, :])
                on_sb.append(t_)
            st["adj"] = adj_sb

            # q = Wq @ ques + bq
            q_ps = pspool.tile([128, 4], F32, tag="ps", name=f"qps_{g}")
            for m, (m0, m1) in enumerate(MS):
                for k in range(4):
                    nc.tensor.matmul(
                        q_ps[:, m : m + 1],
                        wq_sb[k][:, m0:m1],
                        ques_sb[:, k : k + 1],
                        start=(k == 0),
                        stop=(k == 3),
                    )
            q_sb = spool.tile([128, 4], F32, tag="q", name=f"q_{g}")
            for m in range(4):
                nc.vector.tensor_scalar_add(
                    q_sb[:, m : m + 1], q_ps[:, m : m + 1], bq_sb[:, m : m + 1]
                )

            # fold q into We, Wv
            weq_sb, wvq_sb = [], []
            for k in range(4):
                t_ = gpool3.tile([128, H], BF, tag=f"weq{k}", name=f"weq{k}_{g}")
                nc.scalar.activation(
                    out=t_[:], in_=we_sb[k][:], func=ACTF.Copy,
                    scale=q_sb[:, k : k + 1],
                )
                weq_sb.append(t_)
            for k in range(4):
                t_ = gpool3.tile([128, H], BF, tag=f"wvq{k}", name=f"wvq{k}_{g}")
                nc.scalar.activation(
                    out=t_[:], in_=wv_sb[k][:], func=ACTF.Copy,
                    scale=q_sb[:, k : k + 1],
                )
                wvq_sb.append(t_)
            st["weq"] = weq_sb
            st["wvq"] = wvq_sb

            # transposed on-term: ontT[n, c] = sum_f on[f, n] * W1a[f, c]
            # (lhsT = on chunk, rhs = W1a chunk -- no transpose needed).
            # Packed stage-A operands: one K=125 matmul covers
            # [ontT (80) | b1 (1) | w1b chunk3 (44)] against
            # [smat (80) | ones (1) | adj chunk3 (44)], folding the
            # on-term + bias + ragged D-chunk into a single accumulation.
            ontT_ps = pspool.tile([N, H], F32, tag="ps", name=f"ontTps_{g}")
            for ki in range(3):
                nc.tensor.matmul(
                    ontT_ps[:],
                    on_sb[ki][:],
                    w1a_sb[ki][:],
                    start=(ki == 0),
                    stop=(ki == 2),
                )
            KX = N + 1 + (D - 256)  # 125
            w1x_sb = spool.tile([KX, H], BF, tag="w1x", name=f"w1x_{g}")
            nc.scalar.copy(out=w1x_sb[:N, :], in_=ontT_ps[:])
            nc.sync.dma_start(out=w1x_sb[N : N + 1, :], in_=b1row_d[:, :])
            nc.sync.dma_start(out=w1x_sb[N + 1 :, :], in_=w1b_d[256:D, :])
            adjx_sb = gpool3.tile([KX, TOK], BF, tag="adjx", name=f"adjx_{g}")
            nc.sync.dma_start(out=adjx_sb[: N + 1, :], in_=smat_d[:, :])
            nc.sync.dma_start(out=adjx_sb[N + 1 :, :], in_=adjT[g, 256:D, :])
            st["w1x"] = w1x_sb
            st["adjx"] = adjx_sb
            return st

        def emit_AB(g, st):
            """Stage A/B + softmax-a chain (C/D)."""
            adj_sb = st["adj"]
            w1x_sb = st["w1x"]
            adjx_sb = st["adjx"]
            weq_sb = st["weq"]

            # edges_sb: stage-A edges, later reused for t (stage E output)
            # expa_sb: exp(logits_a), later reused for edges2
            edges_sb = [
                gpool.tile([128, TOK], BF, tag=f"edges{m}", name=f"edges{m}_{g}")
                for m in range(4)
            ]
            expa_sb = [
                gpool.tile([128, TOK], BF, tag=f"expa{m}", name=f"expa{m}_{g}")
                for m in range(4)
            ]
            st["edges"] = edges_sb
            st["expa"] = expa_sb

            # stage A: edges = W1b @ adj + on-term + b1, with the ragged
            # 44-row D-chunk packed together with [ontT | b1] (K=125)
            for m, (m0, m1) in enumerate(MS):
                eps = [
                    pspool.tile([128, T], F32, tag="ps", name=f"eps_{g}_{m}_{t}")
                    for t in range(NT)
                ]
                for ki in range(2):
                    for t in range(NT):
                        nc.tensor.matmul(
                            eps[t][:],
                            w1b_sb[ki][:, m0:m1],
                            adj_sb[ki][:, tsl(t)],
                            start=(ki == 0),
                            stop=False,
                        )
                for t in range(NT):
                    nc.tensor.matmul(
                        eps[t][:],
                        w1x_sb[:, m0:m1],
                        adjx_sb[:, tsl(t)],
                        start=False,
                        stop=True,
                    )
                for t in range(NT):
                    nc.scalar.copy(
                        out=edges_sb[m][:, tsl(t)], in_=eps[t][:]
                    )

            # stage B: expa = exp(We' @ edges + be)
            for m, (m0, m1) in enumerate(MS):
                lps = [
                    pspool.tile([128, T], F32, tag="ps", name=f"lps_{g}_{m}_{t}")
                    for t in range(NT)
                ]
                for k in range(4):
                    for t in range(NT):
                        nc.tensor.matmul(
                            lps[t][:],
                            weq_sb[k][:, m0:m1],
                            edges_sb[k][:, tsl(t)],
                            start=(k == 0),
                            stop=(k == 3),
                        )
                for t in range(NT):
                    nc.scalar.activation(
                        out=expa_sb[m][:, tsl(t)],
                        in_=lps[t][:],
                        func=ACTF.Exp,
                        bias=be_sb[:, m : m + 1],
                    )

            # stage C: suma, reca
            reca_sb = []
            for m in range(4):
                suma = spool.tile([128, N], F32, tag=f"suma{m}", name=f"suma{m}_{g}")
                nc.vector.tensor_reduce(
                    suma[:], _re3(expa_sb[m][:]), axis=AX.X, op=ALU.add
                )
                reca = spool.tile([128, N], F32, tag=f"reca{m}", name=f"reca{m}_{g}")
                nc.vector.reciprocal(reca[:], suma[:])
                reca_sb.append(reca)

            # stage D: t1 = expa*edges (in place into edges_sb);
            # edges2 = t1*reca (into expa_sb)
            for m in range(4):
                nc.vector.tensor_tensor(
                    out=edges_sb[m][:],
                    in0=expa_sb[m][:],
                    in1=edges_sb[m][:],
                    op=ALU.mult,
                )
                nc.gpsimd.tensor_tensor(
                    out=_re3(expa_sb[m][:]),
                    in0=_re3(edges_sb[m][:]),
                    in1=reca_sb[m][:, :, None].broadcast_to((128, N, E)),
                    op=ALU.mult,
                )
            return st

        def emit_EF(g, st):
            """Stages E and F for group g."""
            adj_sb = st["adj"]
            edges_sb = st["edges"]   # will hold t
            expa_sb = st["expa"]     # holds edges2
            wvq_sb = st["wvq"]

            expb_sb = [
                gpool.tile([128, TOK], BF, tag=f"expb{m}", name=f"expb{m}_{g}")
                for m in range(4)
            ]
            st["expb"] = expb_sb

            # stage E: t = W2a @ adj + W2b @ edges2 + b2 (into edges_sb)
            for m, (m0, m1) in enumerate(MS):
                tps = [
                    pspool.tile([128, T], F32, tag="ps", name=f"tps_{g}_{m}_{t}")
                    for t in range(NT)
                ]
                for ki in range(3):
                    for t in range(NT):
                        nc.tensor.matmul(
                            tps[t][:],
                            w2a_sb[ki][:, m0:m1],
                            adj_sb[ki][:, tsl(t)],
                            start=(ki == 0),
                            stop=False,
                        )
                for k in range(4):
                    for t in range(NT):
                        nc.tensor.matmul(
                            tps[t][:],
                            w2b_sb[k][:, m0:m1],
                            expa_sb[k][:, tsl(t)],
                            start=False,
                            stop=(k == 3),
                        )
                for t in range(NT):
                    nc.scalar.activation(
                        out=edges_sb[m][:, tsl(t)],
                        in_=tps[t][:],
                        func=ACTF.Identity,
                        bias=b2_sb[:, m : m + 1],
                    )

            # stage F: expb = exp(Wv' @ t + bv)
            for m, (m0, m1) in enumerate(MS):
                bps = [
                    pspool.tile([128, T], F32, tag="ps", name=f"bps_{g}_{m}_{t}")
                    for t in range(NT)
                ]
                for k in range(4):
                    for t in range(NT):
                        nc.tensor.matmul(
                            bps[t][:],
                            wvq_sb[k][:, m0:m1],
                            edges_sb[k][:, tsl(t)],
                            start=(k == 0),
                            stop=(k == 3),
                        )
                for t in range(NT):
                    nc.scalar.activation(
                        out=expb_sb[m][:, tsl(t)],
                        in_=bps[t][:],
                        func=ACTF.Exp,
                        bias=bv_sb[:, m : m + 1],
                    )

        def emit_GHI(g, st):
            """Stages G..I for group g."""
            adj_sb = st["adj"]
            expb_sb = st["expb"]

            # stage G: sumb, recb
            recb_sb = []
            for m in range(4):
                sumb = spool.tile([128, N], F32, tag=f"sumb{m}", name=f"sumb{m}_{g}")
                nc.vector.tensor_reduce(
                    sumb[:], _re3(expb_sb[m][:]), axis=AX.X, op=ALU.add
                )
                recb = spool.tile([128, N], F32, tag=f"recb{m}", name=f"recb{m}_{g}")
                nc.vector.reciprocal(recb[:], sumb[:])
                recb_sb.append(recb)

            # stage H: pre = (Wadj @ adj + badj) * expb  (into expb_sb)
            for m, (m0, m1) in enumerate(MS):
                aps = [
                    pspool.tile([128, T], F32, tag="ps", name=f"aps_{g}_{m}_{t}")
                    for t in range(NT)
                ]
                for ki in range(3):
                    for t in range(NT):
                        nc.tensor.matmul(
                            aps[t][:],
                            wadj_sb[ki][:, m0:m1],
                            adj_sb[ki][:, tsl(t)],
                            start=(ki == 0),
                            stop=(ki == 2),
                        )
                for t in range(NT):
                    nc.vector.scalar_tensor_tensor(
                        out=expb_sb[m][:, tsl(t)],
                        in0=aps[t][:],
                        scalar=badj_sb[:, m : m + 1],
                        in1=expb_sb[m][:, tsl(t)],
                        op0=ALU.add,
                        op1=ALU.mult,
                    )

            # stage I: out = (sum_E pre) * recb ; store
            for m in range(4):
                s_sb = spool.tile([128, N], F32, tag=f"s{m}", name=f"s{m}_{g}")
                nc.vector.tensor_reduce(
                    s_sb[:], _re3(expb_sb[m][:]), axis=AX.X, op=ALU.add
                )
                o_sb = spool.tile([128, N], F32, tag=f"o{m}", name=f"o{m}_{g}")
                nc.gpsimd.tensor_tensor(
                    out=o_sb[:], in0=s_sb[:], in1=recb_sb[m][:], op=ALU.mult
                )
                nc.sync.dma_start(out=outT[g, m, :, :], in_=o_sb[:])

        # software pipeline across groups, 3 stages deep:
        #   ... AB(g) | EF(g-1) | preamble(g+1) | GHI(g-1) ...
        # The serial preamble chain (q -> weight folds -> on-term expand) of
        # g+1 is tucked after stage F's ACT work so it never sits between
        # stage-critical ACT/DVE ops, and completes long before AB(g+1).
        states = {0: emit_preamble(0), 1: emit_preamble(1)}
        for g in range(G):
            emit_AB(g, states[g])
            if g >= 1:
                emit_EF(g - 1, states[g - 1])
            if g >= 1 and g + 1 < G:
                states[g + 1] = emit_preamble(g + 1)
            if g >= 1:
                emit_GHI(g - 1, states.pop(g - 1))
        emit_EF(G - 1, states[G - 1])
        emit_GHI(G - 1, states.pop(G - 1))

    nsplit = _split_multi_waits(nc)
    if os.environ.get("KERNEL_DEBUG"):
        print(f"split_multi_waits: {nsplit} nops inserted", file=sys.stderr)
    return nc


def _pack_bias(b):
    # [H] -> [128, 4]: column j = channels j*128..(j+1)*128
    return np.ascontiguousarray(np.asarray(b, np.float32).reshape(4, 128).T)


def _bf(x):
    return np.ascontiguousarray(np.asarray(x, np.float32).astype(ml_dtypes.bfloat16))


def _smat():
    """[N+1, TOK] node->token selection matrix (+ ones row for the b1 bias)."""
    s = np.zeros((N + 1, TOK), np.float32)
    for n in range(N):
        s[n, n * E : (n + 1) * E] = 1.0
    s[N, :] = 1.0
    return _bf(s)


def prepare_inputs(ques_embed, adj_list, original_nodes,
                   w1_w, w1_b, wq_w, wq_b, we_w, we_b,
                   w2_w, w2_b, wv_w, wv_b, wadj_w, wadj_b):
    """Host-side layout prep: all tensors feature-major bf16, plus per-core
    shards. Returns a list of per-core input maps."""
    adjT = _bf(
        np.asarray(adj_list, np.float32).reshape(BR, TOK, D).transpose(0, 2, 1)
    )
    onT = _bf(
        np.asarray(original_nodes, np.float32).reshape(BR, N, D).transpose(0, 2, 1)
    )
    quesT = _bf(
        np.asarray(ques_embed, np.float32).reshape(BR, 4, 128).transpose(0, 2, 1)
    )

    w = {
        "w1a": _bf(np.asarray(w1_w)[:, :D].T),
        "w1b": _bf(np.asarray(w1_w)[:, D:].T),
        "w2a": _bf(np.asarray(w2_w)[:, :D].T),
        "w2b": _bf(np.asarray(w2_w)[:, D:].T),
        "wadj": _bf(np.asarray(wadj_w).T),
        "wq": _bf(np.asarray(wq_w).T),
        "we": _bf(np.asarray(we_w).T),
        "wv": _bf(np.asarray(wv_w).T),
        "bq": _pack_bias(wq_b),
        "b1row": _bf(np.asarray(w1_b, np.float32).reshape(1, H)),
        "smat": _smat(),
        "be": _pack_bias(we_b),
        "b2": _pack_bias(w2_b),
        "bv": _pack_bias(wv_b),
        "badj": _pack_bias(wadj_b),
    }

    in_maps = []
    for c in range(NCORES):
        sl = slice(c * G, (c + 1) * G)
        m = dict(w)
        m["adjT"] = np.ascontiguousarray(adjT[sl])
        m["onT"] = np.ascontiguousarray(onT[sl])
        m["quesT"] = np.ascontiguousarray(quesT[sl])
        in_maps.append(m)
    return in_maps


def run(in_maps, trace=False, tmpdir=None):
    _install_ntff_hook()
    if not os.environ.get("KERNEL_NO_LDW_DEDUPE"):
        _patch_ldw_dedupe()
    from concourse.bass_utils import run_bass_kernel_spmd

    nc = build_program()
    res = run_bass_kernel_spmd(
        nc,
        in_maps,
        core_ids=list(range(NCORES)),
        trace=trace,
        tmpdir=tmpdir,
    )
    return res


def gather_output(res):
    outT = np.stack([res.results[c]["outT"] for c in range(NCORES)])  # [8,5,4,128,N]
    outT = outT.reshape(BR, 4, 128, N).transpose(0, 3, 1, 2)          # [40,N,4,128]
    return np.ascontiguousarray(outT.reshape(B, R, N, H).astype(np.float32))


def kernel(ques_embed, adj_list, original_nodes,
           w1_w, w1_b, wq_w, wq_b, we_w, we_b,
           w2_w, w2_b, wv_w, wv_b, wadj_w, wadj_b,
           deg=None, batch_size=None, **_unused):
    in_maps = prepare_inputs(
        ques_embed, adj_list, original_nodes,
        w1_w, w1_b, wq_w, wq_b, we_w, we_b,
        w2_w, w2_b, wv_w, wv_b, wadj_w, wadj_b,
    )
    res = run(in_maps, trace=False)
    return gather_output(res)

